# revision 1
# baseline (speedup 1.0000x reference)
"""Trainium2 Bass kernel for KeyChannelwiseMemoryMultiHead.

Math: for each pixel vector x (256):
  y1 = w_in @ x + b_in                      (512 = 64 key x 8 heads, chan = k*8+n)
  a[n,m] = sum_k key_p[n,k,m] * y1[k*8+n]   (per-head key matmul)
  s = softmax_m(a[n,:])
  z[n,d] = sum_m memory[n,m,d] * s[n,m]
  out = w_out @ z_flat + b_out              (z chan = n*64+d)

Host-side exact refactor (fp64 weight folding):
  KW[(n,m), c] = sum_k key_p[n,k,m] w_in[k*8+n, c]   -> stage A: A = KW @ x
  kb[(n,m)]    = sum_k key_p[n,k,m] b_in[k*8+n]
  WM[o, (n,m)] = (sum_d w_out[o, n*64+d] memory[n,m,d]) * exp(kb[(n,m)])
  softmax(A + kb) folded:  E = exp(A);  wsum[n] = sum_m exp(kb) E;  S^ = E / wsum
  out = WM @ S^ + b_out

On-chip (per core = one batch, pixels chunked by 512):
  stage A: 2 K-tile matmuls -> PSUM [128 nm, 512 pix]  (4 nm tiles)
  exp:     ScalarE activation PSUM->SBUF
  wsum:    matmul with block-diagonal [128,128] (ekb-weighted head-indicator)
           -> per-head sums already broadcast across the 128 partitions
  recip:   VectorE PSUM->SBUF;  S^ = E * R  (VectorE)
  stage B: 8 accumulating matmuls -> PSUM [128 out, 512 pix] (2 o tiles x 4 K)
  bias:    VectorE tensor_scalar_add, DMA out.
"""

import os
import sys

import numpy as np

for _p in ("/opt/trn_rl_repo", "/root/.axon_site/_ro/trn_rl_repo"):
    if os.path.isdir(_p) and _p not in sys.path:
        sys.path.insert(0, _p)

import concourse.bass as bass  # noqa: E402
import concourse.tile as tile  # noqa: E402
from concourse import bacc, bass_utils, mybir  # noqa: E402
from concourse import dve_ops as _dve_ops  # noqa: E402
from concourse.dve_spec import (  # noqa: E402
    AluOp,
    Bin,
    C0,
    C1,
    Spec,
    Src0,
    Src1,
    _has_src1,
    lower,
)
from concourse.dve_table_gen import dve_ver_for  # noqa: E402
from concourse.dve_uop import DveOpSpec  # noqa: E402

N_CORES = 8
C_IN = 256
NM = 512  # heads * mem_dim, channel order (n outer, m inner)
C_OUT = 256
NPIX = 64 * 64
CHUNK = 512
N_CHUNKS = NPIX // CHUNK
FP32 = mybir.dt.float32
FP32R = mybir.dt.float32r
BF16 = mybir.dt.bfloat16
# Chebyshev seed constants shared with RECIPROCAL_APPROX_FAST; after ONE
# Newton step the recip rel-err is balanced at ~1.7e-3 (minimax pair).
_RC0 = -0.23549792
_RC1 = 2.0017324

_FUSED_OP = None


def _register_fused_divmul():
    """out = in1 * approx_recip(in0): BITWISE_NOT exponent-flip seed +
    one inline Newton pass + multiply by in1 -- single DVE pass replacing
    reciprocal()+tensor_mul() on the softmax normalization path."""
    global _FUSED_OP
    if _FUSED_OP is not None:
        return _FUSED_OP
    name = "RECIP1NR_MUL_ANT"
    _not_x = Bin(AluOp.BITWISE_NOT, Src0, Src0)
    _y0 = _not_x * C0
    _y1 = _y0 * (C1 - Src0 * _y0)

    def _ref(in0, in1, c0, c1, c2):
        not_x = (~in0.view(np.int32)).view(np.float32)
        y0 = not_x * c0
        y1 = y0 * (c1 - in0 * y0)
        return y1 * in1

    spec = Spec(body=_y1 * Src1, reference=_ref)
    row = max(_dve_ops._SUB_OPCODE_FOR_NAME.values()) + 1
    assert row < 0x20
    _dve_ops._SUB_OPCODE_FOR_NAME[name] = row
    shas = {}
    for ver in ("v3",):
        s = DveOpSpec(name=name, opcode=row, uops=lower(spec, ver=ver),
                      rd1_en=_has_src1(spec))
        shas[ver] = s.sha(ver)
    op = _dve_ops.DveOp(name, spec, subdim=False, uops_sha=shas)
    _dve_ops.OPS.append(op)
    _dve_ops.CUSTOM_DVE_SPECS[name] = spec
    _FUSED_OP = op
    return op

_CACHED_NC = None


def _build_nc():
    nc = bacc.Bacc(
        "TRN2",
        target_bir_lowering=False,
        debug=False,
        enable_asserts=True,
        num_devices=N_CORES,
    )
    x_d = nc.dram_tensor("x", [C_IN, NPIX], BF16, kind="ExternalInput")
    kwt_d = nc.dram_tensor("kwt", [C_IN, NM], BF16, kind="ExternalInput")
    sumw_d = nc.dram_tensor("sumw", [128, NM], BF16, kind="ExternalInput")
    wmt_d = nc.dram_tensor("wmt", [NM, C_OUT], BF16, kind="ExternalInput")
    bout_d = nc.dram_tensor("bout", [128, 2], FP32, kind="ExternalInput")
    out_d = nc.dram_tensor("out", [C_OUT, NPIX], FP32, kind="ExternalOutput")

    Exp = mybir.ActivationFunctionType.Exp
    fused = _register_fused_divmul()

    with tile.TileContext(nc) as tc:
        with (
            tc.tile_pool(name="wpool", bufs=1) as wpool,
            tc.tile_pool(name="xpool", bufs=3) as xpool,
            tc.tile_pool(name="epool", bufs=4) as epool,
            tc.tile_pool(name="rpool", bufs=3) as rpool,
            tc.tile_pool(name="spool", bufs=4) as spool,
            tc.tile_pool(name="opool", bufs=3) as opool,
            tc.tile_pool(name="pa", bufs=2, space="PSUM") as pa,
            tc.tile_pool(name="ps", bufs=2, space="PSUM") as ps,
            tc.tile_pool(name="po", bufs=2, space="PSUM") as po,
        ):
            kwt = []
            for i in range(2):
                t_ = wpool.tile([128, NM], BF16, name=f"kwt{i}", tag=f"kwt{i}")
                nc.sync.dma_start(t_[:], kwt_d[i * 128 : (i + 1) * 128, :])
                kwt.append(t_)
            sumw = wpool.tile([128, NM], BF16, name="sumw", tag="sumw")
            nc.sync.dma_start(sumw[:], sumw_d[:, :])
            wmt = []
            for t in range(4):
                t_ = wpool.tile([128, C_OUT], BF16, name=f"wmt{t}", tag=f"wmt{t}")
                nc.sync.dma_start(t_[:], wmt_d[t * 128 : (t + 1) * 128, :])
                wmt.append(t_)
            bout = wpool.tile([128, 2], FP32, name="bout", tag="bout")
            nc.sync.dma_start(bout[:], bout_d[:, :])

            for j in range(N_CHUNKS):
                xc = []
                for i in range(2):
                    t_ = xpool.tile([128, CHUNK], BF16, name=f"x{i}_{j}", tag=f"x{i}")
                    nc.sync.dma_start(
                        t_[:], x_d[i * 128 : (i + 1) * 128, j * CHUNK : (j + 1) * CHUNK]
                    )
                    xc.append(t_)

                s_tiles = []
                for t in range(4):
                    a_ps = pa.tile([128, CHUNK], FP32, name=f"pa_{j}_{t}", tag="pa")
                    for i in range(2):
                        nc.tensor.matmul(
                            a_ps[:],
                            kwt[i][:, t * 128 : (t + 1) * 128],
                            xc[i][:],
                            start=(i == 0),
                            stop=(i == 1),
                        )
                    e_sb = epool.tile([128, CHUNK], BF16, name=f"e_{j}_{t}", tag="e")
                    nc.scalar.activation(e_sb[:], a_ps[:], Exp)
                    s_ps = ps.tile([128, CHUNK], FP32, name=f"ps_{j}_{t}", tag="ps")
                    nc.tensor.matmul(
                        s_ps[:],
                        sumw[:, t * 128 : (t + 1) * 128],
                        e_sb[:],
                        start=True,
                        stop=True,
                    )
                    s_sb = spool.tile([128, CHUNK], BF16, name=f"s_{j}_{t}", tag="s")
                    nc.vector._custom_dve(
                        fused,
                        out=s_sb[:],
                        in0=s_ps[:],
                        in1=e_sb[:],
                        s0=_RC0,
                        s1=_RC1,
                    )
                    s_tiles.append(s_sb)

                po_t = [po.tile([128, CHUNK], FP32, name=f"po{o}_{j}", tag=f"po{o}") for o in range(2)]
                for t in range(4):
                    for o in range(2):
                        nc.tensor.matmul(
                            po_t[o][:],
                            wmt[t][:, o * 128 : (o + 1) * 128],
                            s_tiles[t][:],
                            start=(t == 0),
                            stop=(t == 3),
                        )
                for o in range(2):
                    o_sb = opool.tile([128, CHUNK], FP32, name=f"o{o}_{j}", tag=f"o{o}")
                    if o == 0:
                        nc.scalar.activation(
                            o_sb[:],
                            po_t[o][:],
                            mybir.ActivationFunctionType.Identity,
                            bias=bout[:, o : o + 1],
                        )
                    else:
                        nc.vector.tensor_scalar_add(
                            o_sb[:], po_t[o][:], bout[:, o : o + 1]
                        )
                    nc.sync.dma_start(
                        out_d[o * 128 : (o + 1) * 128, j * CHUNK : (j + 1) * CHUNK],
                        o_sb[:],
                    )

    nc.compile()
    return nc


def _fold_weights(key_p, memory, w_in, b_in, w_out, b_out):
    key_p = np.asarray(key_p, np.float64)
    memory = np.asarray(memory, np.float64)
    w_in = np.asarray(w_in, np.float64)
    b_in = np.asarray(b_in, np.float64)
    w_out = np.asarray(w_out, np.float64)
    b_out = np.asarray(b_out, np.float64)

    w_in_r = w_in.reshape(64, 8, C_IN)  # [k, n, c]
    kw = np.einsum("nkm,knc->nmc", key_p, w_in_r)  # [n, m, c]
    import ml_dtypes

    kwt = np.ascontiguousarray(kw.reshape(NM, C_IN).T.astype(ml_dtypes.bfloat16))

    kb = np.einsum("nkm,kn->nm", key_p, b_in.reshape(64, 8))  # [n, m]
    ekb = np.exp(kb).reshape(NM)  # (n,m) flat

    w_out_r = w_out.reshape(C_OUT, 8, 64)  # [o, n, d]
    wm = np.einsum("ond,nmd->onm", w_out_r, memory)  # [o, n, m]
    wmp = wm.reshape(C_OUT, NM) * ekb[None, :]
    wmt = np.ascontiguousarray(wmp.T.astype(ml_dtypes.bfloat16))

    sumw = np.zeros((128, NM), ml_dtypes.bfloat16)
    for t in range(4):
        ekb_t = ekb[128 * t : 128 * (t + 1)]
        blk = np.zeros((128, 128))
        blk[:64, :64] = ekb_t[:64, None]
        blk[64:, 64:] = ekb_t[64:, None]
        sumw[:, 128 * t : 128 * (t + 1)] = blk

    bout = np.ascontiguousarray(b_out.reshape(2, 128).T.astype(np.float32))
    return kwt, sumw, wmt, bout


import ml_dtypes as _mld

_ml_bf16 = _mld.bfloat16


def kernel_with_results(trace=False, tmpdir=None, **inputs):
    global _CACHED_NC
    x = np.asarray(inputs["x"], np.float32)  # [8, 256, 64, 64]
    kwt, sumw, wmt, bout = _fold_weights(
        inputs["key_p"],
        inputs["memory"],
        inputs["w_in"],
        inputs["b_in"],
        inputs["w_out"],
        inputs["b_out"],
    )
    if _CACHED_NC is None:
        _CACHED_NC = _build_nc()
    nc = _CACHED_NC

    in_maps = [
        {
            "x": np.ascontiguousarray(
                x[b].reshape(C_IN, NPIX).astype(_ml_bf16)
            ),
            "kwt": kwt,
            "sumw": sumw,
            "wmt": wmt,
            "bout": bout,
        }
        for b in range(N_CORES)
    ]
    res = bass_utils.run_bass_kernel_spmd(
        nc, in_maps, core_ids=list(range(N_CORES)), trace=trace, tmpdir=tmpdir
    )
    out = np.stack(
        [res.results[b]["out"].reshape(C_OUT, 64, 64) for b in range(N_CORES)]
    ).astype(np.float32)
    return out, res


def kernel(**inputs):
    out, _ = kernel_with_results(trace=False, **inputs)
    return out



# revision 4
# speedup vs baseline: 1.2051x; 1.2051x over previous
"""Trainium2 Bass kernel for KeyChannelwiseMemoryMultiHead.

Math: for each pixel vector x (256):
  y1 = w_in @ x + b_in                      (512 = 64 key x 8 heads, chan = k*8+n)
  a[n,m] = sum_k key_p[n,k,m] * y1[k*8+n]   (per-head key matmul)
  s = softmax_m(a[n,:])
  z[n,d] = sum_m memory[n,m,d] * s[n,m]
  out = w_out @ z_flat + b_out              (z chan = n*64+d)

Host-side exact refactor (fp64 weight folding):
  KW[(n,m), c] = sum_k key_p[n,k,m] w_in[k*8+n, c]   -> stage A: A = KW @ x
  kb[(n,m)]    = sum_k key_p[n,k,m] b_in[k*8+n]      -> folded as exp bias
  WM[o, (n,m)] = sum_d w_out[o, n*64+d] memory[n,m,d]
  E = exp(A + kb);  wsum[n] = sum_m E;  S = E / wsum
  out = WM @ S + b_out

On-chip (per core = one batch, pixels chunked by 512):
  stage A: 2 K-tile matmuls -> PSUM [128 nm, 512 pix]  (4 nm tiles)
  exp:     ScalarE activation(Exp, bias=kb) PSUM->SBUF
  wsum:    matmul with block-diagonal ones [128,128] (head-indicator)
           -> per-head sums broadcast across the 128 partitions
  recip:   fused DVE op S = E * approx_recip(wsum)  PSUM->SBUF
  stage B: 8 accumulating matmuls -> PSUM [128 out, 512 pix] (2 o tiles x 4 K)
  bias:    ScalarE identity+bias (o=0) / DVE tensor_scalar_add (o=1), bf16 out.

Perf structure (v2):
  - All weights packed into 2 bf16 DMA blobs + 1 small fp32 blob; x and out
    packed so each chunk is ONE [128,1024] DMA (each HWDGE dma_start costs
    ~625ns on a shared device, so trigger count dominates startup latency).
  - Tensor-engine issue order per chunk j: A(j) | wsum(j,t0,t1) | B(j-1) |
    wsum(j,t2,t3) -- hides the exp/recip latency behind B of the previous
    chunk and lets PSUM recycle with pa=4/ps=2/po=2 banks.
  - Output stored bf16 (host converts to fp32).
"""

import os
import sys

import numpy as np

for _p in ("/opt/trn_rl_repo", "/root/.axon_site/_ro/trn_rl_repo"):
    if os.path.isdir(_p) and _p not in sys.path:
        sys.path.insert(0, _p)

import concourse.bass as bass  # noqa: E402
import concourse.tile as tile  # noqa: E402
from concourse import bacc, bass_utils, mybir  # noqa: E402
from concourse import dve_ops as _dve_ops  # noqa: E402
from concourse.dve_spec import (  # noqa: E402
    AluOp,
    Bin,
    C0,
    C1,
    Spec,
    Src0,
    Src1,
    _has_src1,
    lower,
)
from concourse.dve_table_gen import dve_ver_for  # noqa: E402
from concourse.dve_uop import DveOpSpec  # noqa: E402

N_CORES = 8
C_IN = 256
NM = 512  # heads * mem_dim, channel order (n outer, m inner)
C_OUT = 256
NPIX = 64 * 64
CHUNK = 512
N_CHUNKS = NPIX // CHUNK
FP32 = mybir.dt.float32
BF16 = mybir.dt.bfloat16
# Chebyshev seed constants; after ONE Newton step the recip rel-err is
# balanced at ~1.7e-3 (minimax pair).
_RC0 = -0.23549792
_RC1 = 2.0017324

_FUSED_OP = None


def _register_fused_divmul():
    """out = in1 * approx_recip(in0): BITWISE_NOT exponent-flip seed +
    one inline Newton pass + multiply by in1 -- single DVE pass replacing
    reciprocal()+tensor_mul() on the softmax normalization path."""
    global _FUSED_OP
    if _FUSED_OP is not None:
        return _FUSED_OP
    name = "RECIP1NR_MUL_ANT"
    _not_x = Bin(AluOp.BITWISE_NOT, Src0, Src0)
    _y0 = _not_x * C0
    _y1 = _y0 * (C1 - Src0 * _y0)

    def _ref(in0, in1, c0, c1, c2):
        not_x = (~in0.view(np.int32)).view(np.float32)
        y0 = not_x * c0
        y1 = y0 * (c1 - in0 * y0)
        return y1 * in1

    spec = Spec(body=_y1 * Src1, reference=_ref)
    row = max(_dve_ops._SUB_OPCODE_FOR_NAME.values()) + 1
    assert row < 0x20
    _dve_ops._SUB_OPCODE_FOR_NAME[name] = row
    shas = {}
    for ver in ("v3",):
        s = DveOpSpec(name=name, opcode=row, uops=lower(spec, ver=ver),
                      rd1_en=_has_src1(spec))
        shas[ver] = s.sha(ver)
    op = _dve_ops.DveOp(name, spec, subdim=False, uops_sha=shas)
    _dve_ops.OPS.append(op)
    _dve_ops.CUSTOM_DVE_SPECS[name] = spec
    _FUSED_OP = op
    return op

_CACHED_NC = None


def _build_nc():
    nc = bacc.Bacc(
        "TRN2",
        target_bir_lowering=False,
        debug=False,
        enable_asserts=True,
        num_devices=N_CORES,
    )
    # wd cols: [kwt k0 512 | kwt k1 512 | sumw 128 | wmt 4x256]
    wd_d = nc.dram_tensor("wd", [128, 2176], BF16, kind="ExternalInput")
    # wf cols: [kb tile0..3 | bout o0 | bout o1]
    wf_d = nc.dram_tensor("wf", [128, 6], FP32, kind="ExternalInput")
    # xd row p: [j=0..7][i=0..1][c=0..511] = x[i*128+p, j*512+c]
    xd_d = nc.dram_tensor("xd", [128, 8192], BF16, kind="ExternalInput")
    # od row p: [j=0..7][o=0..1][pix 512] = out[o*128+p, j*512+pix]
    od_d = nc.dram_tensor("od", [128, 8192], BF16, kind="ExternalOutput")

    Exp = mybir.ActivationFunctionType.Exp
    Ident = mybir.ActivationFunctionType.Identity
    fused = _register_fused_divmul()

    with tile.TileContext(nc) as tc:
        with (
            tc.tile_pool(name="wpool", bufs=1) as wpool,
            tc.tile_pool(name="xpool", bufs=3) as xpool,
            tc.tile_pool(name="epool", bufs=5) as epool,
            tc.tile_pool(name="spool", bufs=6) as spool,
            tc.tile_pool(name="opool", bufs=3) as opool,
            tc.tile_pool(name="pa", bufs=4, space="PSUM") as pa,
            tc.tile_pool(name="ps", bufs=2, space="PSUM") as ps,
            tc.tile_pool(name="po", bufs=1, space="PSUM") as po,
        ):
            # --- weight + first-x DMAs, latency-ordered ---
            kw = wpool.tile([128, 1024], BF16, name="kw", tag="kw")
            nc.sync.dma_start(kw[:], wd_d[:, 0:1024])

            xt = {}
            def load_x(j):
                t_ = xpool.tile([128, 1024], BF16, name=f"x{j}", tag="x")
                nc.sync.dma_start(t_[:], xd_d[:, j * 1024 : (j + 1) * 1024])
                xt[j] = t_

            load_x(0)
            wrest = wpool.tile([128, 1152], BF16, name="wrest", tag="wrest")
            nc.sync.dma_start(wrest[:], wd_d[:, 1024:2176])
            wf = wpool.tile([128, 6], FP32, name="wf", tag="wf")
            nc.sync.dma_start(wf[:], wf_d[:, :])
            load_x(1)
            load_x(2)

            sumw = wrest[:, 0:128]

            # carried state from chunk j-1
            prev = None  # (s_tiles, po_t, j-1)

            def issue_B(state):
                s_tiles, po_t, _ = state
                for t in range(4):
                    for o in range(2):
                        nc.tensor.matmul(
                            po_t[o][:],
                            wrest[:, 128 + t * 256 + o * 128 : 128 + t * 256 + (o + 1) * 128],
                            s_tiles[t][:],
                            start=(t == 0),
                            stop=(t == 3),
                        )

            def issue_tail(state):
                """bias-adds + output DMA for chunk jprev."""
                s_tiles, po_t, jp = state
                o_sb = opool.tile([128, 1024], BF16, name=f"o_{jp}", tag="o")
                nc.scalar.activation(
                    o_sb[:, 0:512], po_t[0][:], Ident, bias=wf[:, 4:5]
                )
                nc.vector.tensor_scalar_add(
                    o_sb[:, 512:1024], po_t[1][:], wf[:, 5:6]
                )
                nc.sync.dma_start(
                    od_d[:, jp * 1024 : (jp + 1) * 1024], o_sb[:]
                )

            for j in range(N_CHUNKS):
                xc = xt.pop(j)
                # ---- stage A: 4 nm-tiles x 2 k-halves ----
                a_ps = []
                for t in range(4):
                    t_ps = pa.tile([128, CHUNK], FP32, name=f"pa_{j}_{t}", tag="pa")
                    for i in range(2):
                        nc.tensor.matmul(
                            t_ps[:],
                            kw[:, i * 512 + t * 128 : i * 512 + (t + 1) * 128],
                            xc[:, i * 512 : (i + 1) * 512],
                            start=(i == 0),
                            stop=(i == 1),
                        )
                    a_ps.append(t_ps)

                # exp (scalar engine), issue all 4 now
                e_sb = []
                for t in range(4):
                    e_ = epool.tile([128, CHUNK], BF16, name=f"e_{j}_{t}", tag="e")
                    nc.scalar.activation(e_[:], a_ps[t][:], Exp, bias=wf[:, t : t + 1])
                    e_sb.append(e_)

                # wsum t0,t1
                s_ps = []
                for t in range(2):
                    p_ = ps.tile([128, CHUNK], FP32, name=f"ps_{j}_{t}", tag="ps")
                    nc.tensor.matmul(p_[:], sumw, e_sb[t][:], start=True, stop=True)
                    s_ps.append(p_)

                # recip t0,t1 (DVE)
                s_tiles = []
                for t in range(2):
                    s_ = spool.tile([128, CHUNK], BF16, name=f"s_{j}_{t}", tag="s")
                    nc.vector._custom_dve(
                        fused, out=s_[:], in0=s_ps[t][:], in1=e_sb[t][:],
                        s0=_RC0, s1=_RC1,
                    )
                    s_tiles.append(s_)

                # stage B of previous chunk fills the exp->wsum latency
                po_t = None
                if prev is not None:
                    issue_B(prev)

                # wsum + recip t2,t3
                for t in range(2, 4):
                    p_ = ps.tile([128, CHUNK], FP32, name=f"ps_{j}_{t}", tag="ps")
                    nc.tensor.matmul(p_[:], sumw, e_sb[t][:], start=True, stop=True)
                    s_ = spool.tile([128, CHUNK], BF16, name=f"s_{j}_{t}", tag="s")
                    nc.vector._custom_dve(
                        fused, out=s_[:], in0=p_[:], in1=e_sb[t][:],
                        s0=_RC0, s1=_RC1,
                    )
                    s_tiles.append(s_)

                if prev is not None:
                    issue_tail(prev)

                po_t = [
                    po.tile([128, CHUNK], FP32, name=f"po{o}_{j}", tag=f"po{o}")
                    for o in range(2)
                ]
                prev = (s_tiles, po_t, j)

                if j + 3 < N_CHUNKS:
                    load_x(j + 3)

            issue_B(prev)
            issue_tail(prev)

    nc.compile()
    return nc


def _fold_weights(key_p, memory, w_in, b_in, w_out, b_out):
    import ml_dtypes

    key_p = np.asarray(key_p, np.float64)
    memory = np.asarray(memory, np.float64)
    w_in = np.asarray(w_in, np.float64)
    b_in = np.asarray(b_in, np.float64)
    w_out = np.asarray(w_out, np.float64)
    b_out = np.asarray(b_out, np.float64)

    w_in_r = w_in.reshape(64, 8, C_IN)  # [k, n, c]
    kw = np.einsum("nkm,knc->nmc", key_p, w_in_r)  # [n, m, c]
    kwt = kw.reshape(NM, C_IN).T  # [c, nm]

    kb = np.einsum("nkm,kn->nm", key_p, b_in.reshape(64, 8)).reshape(NM)

    w_out_r = w_out.reshape(C_OUT, 8, 64)  # [o, n, d]
    wm = np.einsum("ond,nmd->onm", w_out_r, memory)  # [o, n, m]
    wmt = wm.reshape(C_OUT, NM).T  # [nm, o]

    wd = np.zeros((128, 2176), ml_dtypes.bfloat16)
    wd[:, 0:512] = kwt[0:128, :].astype(ml_dtypes.bfloat16)
    wd[:, 512:1024] = kwt[128:256, :].astype(ml_dtypes.bfloat16)
    blk = np.zeros((128, 128))
    blk[:64, :64] = 1.0
    blk[64:, 64:] = 1.0
    wd[:, 1024:1152] = blk.astype(ml_dtypes.bfloat16)
    for t in range(4):
        wd[:, 1152 + t * 256 : 1152 + (t + 1) * 256] = (
            wmt[t * 128 : (t + 1) * 128, :].astype(ml_dtypes.bfloat16)
        )

    wf = np.zeros((128, 6), np.float32)
    wf[:, 0:4] = kb.reshape(4, 128).T
    wf[:, 4:6] = b_out.reshape(2, 128).T
    return np.ascontiguousarray(wd), np.ascontiguousarray(wf)


import ml_dtypes as _mld

_ml_bf16 = _mld.bfloat16


def kernel_with_results(trace=False, tmpdir=None, **inputs):
    global _CACHED_NC
    x = np.asarray(inputs["x"], np.float32)  # [8, 256, 64, 64]
    wd, wf = _fold_weights(
        inputs["key_p"],
        inputs["memory"],
        inputs["w_in"],
        inputs["b_in"],
        inputs["w_out"],
        inputs["b_out"],
    )
    if _CACHED_NC is None:
        _CACHED_NC = _build_nc()
    nc = _CACHED_NC

    in_maps = []
    for b in range(N_CORES):
        xb = x[b].reshape(2, 128, 8, 512)  # [i, p, j, c]
        xd = np.ascontiguousarray(
            xb.transpose(1, 2, 0, 3).reshape(128, 8192).astype(_ml_bf16)
        )
        in_maps.append({"xd": xd, "wd": wd, "wf": wf})

    res = bass_utils.run_bass_kernel_spmd(
        nc, in_maps, core_ids=list(range(N_CORES)), trace=trace, tmpdir=tmpdir
    )
    outs = []
    for b in range(N_CORES):
        od = np.asarray(res.results[b]["od"]).astype(np.float32)
        od = od.reshape(128, 8, 2, 512)  # [p, j, o, c]
        outs.append(od.transpose(2, 0, 1, 3).reshape(C_OUT, 64, 64))
    out = np.stack(outs)
    return out, res


def kernel(**inputs):
    out, _ = kernel_with_results(trace=False, **inputs)
    return out
